# revision 20
# baseline (speedup 1.0000x reference)
"""CornerNet module (2x nonlocal attention + 6 conv heads) on 8 Trainium2 cores.

Sharding: the NxN (8192x8192) attention in each nonlocal block is sharded by
rows across the 8 cores.  Row-group granularity is 128 rows; row i of the
attention corresponds to flat index (b, cb, blk) of the theta buffer, and we
assign core d the blocks blk in [4d, 4d+4) for both batches.  Those blocks map
to image rows [8d, 8d+8) of the y output, so each core also owns a contiguous
spatial slice for the conv heads.  phi / g / the 1x1 convs are computed
replicated (they are tiny).  The 1-row halo needed by the 3x3 head convs is
exchanged with an AllGather of the boundary y rows.

Layout cheat sheet (per side):
  A   = theta flat  [8192, 128]  rows i=(b, cb, blk), col k = hw offset in blk
  P   = phi  flat   [128, 8192]  row k=(b_k, cb//2),  col j=(cb%2)*4096+hw
  G   = g    flat   [8192, 128]  row j, col n;  G[(s,u), n] = g[s, u//32, (u%32)*128+n]
  S^T tile for j-chunk (s, blkg):  lhsT = P[:, s*4096+blkg :: 32]  (cb on M)
                                   rhs  = A^T tile [k, i]          (from theta conv)
  Y accumulation: lhsT = exp(S^T)[:, i-slice], rhs = g_nat[s][:, blkg*128:+128] | ones
Softmax skips max-subtraction: |scores| <= |theta_row| * |phi_col| ~ 15, safe in f32.
"""

import numpy as np
import ml_dtypes

import concourse.bass as bass
import concourse.mybir as mybir
import concourse.tile as tile
from concourse.tile import add_dep_helper
from concourse import bacc
from concourse.bass_utils import run_bass_kernel_spmd

BF16 = mybir.dt.bfloat16
F32 = mybir.dt.float32
I32 = mybir.dt.int32

P = 128          # partitions
NB = 2           # batch
NS = 2           # sides (tl, br)
C = 256          # channels
CB = 128         # bottleneck channels
H = W = 64
HW = H * W       # 4096
NCORES = 8
AF = mybir.ActivationFunctionType


def _build_nc():
    nc = bacc.Bacc(num_devices=NCORES)

    # ---- inputs (all 2D except w1) ----
    xb_d = nc.dram_tensor("xb", [P, 2 * NB * HW], BF16, kind="ExternalInput")
    xown_d = nc.dram_tensor("xown", [P, 2 * NB * 512], BF16, kind="ExternalInput")
    xres_d = nc.dram_tensor("xres", [P, 2 * NB * 640], F32, kind="ExternalInput")
    wth_d = nc.dram_tensor("wth", [P, NS * 2 * CB], BF16, kind="ExternalInput")
    wph_d = nc.dram_tensor("wph", [P, NS * 2 * CB], BF16, kind="ExternalInput")
    wg_d = nc.dram_tensor("wg", [P, NS * 2 * CB], BF16, kind="ExternalInput")
    wW_d = nc.dram_tensor("wW", [P, NS * C], BF16, kind="ExternalInput")
    bthb_d = nc.dram_tensor("bthb", [P, NS * CB], F32, kind="ExternalInput")
    bqkv_d = nc.dram_tensor("bqkv", [P, NS * 2], F32, kind="ExternalInput")  # phi,g
    bW_d = nc.dram_tensor("bW", [P, NS * 2], F32, kind="ExternalInput")
    w1_d = nc.dram_tensor("w1", [NS, 3, P, 9 * 2 * C], BF16, kind="ExternalInput")
    b1_d = nc.dram_tensor("b1", [P, NS * 3 * 2], F32, kind="ExternalInput")
    w2h_d = nc.dram_tensor("w2h", [P, NS * 2 * 80], BF16, kind="ExternalInput")
    b2h_d = nc.dram_tensor("b2h", [P, NS], F32, kind="ExternalInput")
    w2t_d = nc.dram_tensor("w2t", [P, NS * 2 * 1], BF16, kind="ExternalInput")
    b2t_d = nc.dram_tensor("b2t", [P, NS], F32, kind="ExternalInput")
    w2r_d = nc.dram_tensor("w2r", [P, NS * 2 * 2], BF16, kind="ExternalInput")
    b2r_d = nc.dram_tensor("b2r", [P, NS], F32, kind="ExternalInput")
    hidx_d = nc.dram_tensor("hidx", [P, 2], I32, kind="ExternalInput")
    hmask_d = nc.dram_tensor("hmask", [P, 2], F32, kind="ExternalInput")

    # ---- outputs (spatial shard: rows [8d, 8d+8) = 512 pixels per batch) ----
    heat_d = nc.dram_tensor("heat", [NS, NB, 80, 512], F32, kind="ExternalOutput")
    tago_d = nc.dram_tensor("tago", [NS, NB, 1, 512], F32, kind="ExternalOutput")
    regr_d = nc.dram_tensor("regr", [NS, NB, 2, 512], F32, kind="ExternalOutput")

    with tile.TileContext(nc) as tc:
        with (
            tc.tile_pool(name="const", bufs=1) as const,
            tc.tile_pool(name="pers", bufs=1) as pers,
            tc.tile_pool(name="phin", bufs=2) as phin,
            tc.tile_pool(name="stp", bufs=3) as stp,
            tc.tile_pool(name="w1p", bufs=2) as w1p,
            tc.tile_pool(name="hp", bufs=2) as hp,
            tc.tile_pool(name="f32p", bufs=2) as f32p,
            tc.tile_pool(name="obp", bufs=2) as obp,
            tc.tile_pool(name="smal", bufs=4) as smal,
            tc.tile_pool(name="py", bufs=1, space="PSUM") as ppy,
            tc.tile_pool(name="ps4", bufs=1, space="PSUM") as pps,
            tc.tile_pool(name="st4", bufs=4) as stp4,
            tc.tile_pool(name="dram", bufs=1, space="DRAM") as dram,
        ):
            _cvstate = [0]
            _cv4tags = ["yt0", "yt1", "dn0", "dn1"]

            def cvtile(shape):
                _cvstate[0] ^= 1
                return ppy.tile(shape, F32, tag=f"yt{_cvstate[0]}", name="cvt")

            def hxtile(shape):
                return ppy.tile(shape, F32, tag="hx", name="hx")

            def dntile(shape, dtype=F32):
                _cvstate[0] ^= 1
                return ppy.tile(shape, dtype, tag=f"dn{_cvstate[0]}", name="dnt")

            # ---- load inputs into SBUF ----
            def load(d, dtype, tag):
                t = const.tile(list(d.shape), dtype, tag=tag, name=tag)
                nc.sync.dma_start(t[:], d.ap())
                return t

            wth_sb = load(wth_d, BF16, "wth")
            xown_sb = load(xown_d, BF16, "xown")
            xb_sb = const.tile(list(xb_d.shape), BF16, tag="xb", name="xb")
            for b_ in range(NB):
                for ck_ in range(2):
                    o_ = (ck_ * NB + b_) * HW
                    nc.sync.dma_start(xb_sb[:, o_:o_ + HW], xb_d.ap()[:, o_:o_ + HW])
            wph_sb = load(wph_d, BF16, "wph")
            wg_sb = load(wg_d, BF16, "wg")
            wW_sb = load(wW_d, BF16, "wW")
            bthb_sb = load(bthb_d, F32, "bthb")
            bqkv_sb = load(bqkv_d, F32, "bqkv")
            bW_sb = load(bW_d, F32, "bW")
            b1_sb = load(b1_d, F32, "b1")
            w2h_sb = load(w2h_d, BF16, "w2h")
            b2h_sb = load(b2h_d, F32, "b2h")
            w2t_sb = load(w2t_d, BF16, "w2t")
            b2t_sb = load(b2t_d, F32, "b2t")
            w2r_sb = load(w2r_d, BF16, "w2r")
            b2r_sb = load(b2r_d, F32, "b2r")
            hidx_sb = load(hidx_d, I32, "hidx")
            hmask_sb = load(hmask_d, F32, "hmask")
            xres_sb = load(xres_d, F32, "xres")

            # persistent per-side tensors
            P_sb = [pers.tile([P, 2 * HW], BF16, tag=f"P{s}", name=f"P{s}") for s in range(NS)]
            gnat = [pers.tile([P, 64 * 128], BF16, tag=f"g{s}", name=f"g{s}") for s in range(NS)]
            A_sb = [pers.tile([P, 8 * CB], BF16, tag=f"A{s}", name=f"A{s}") for s in range(NS)]
            ysh = [pers.tile([P, 8 * CB], BF16, tag=f"y{s}", name=f"ysh{s}") for s in range(NS)]
            nl_sb = [pers.tile([P, 4 * 10 * 66], BF16, tag=f"nl{s}", name=f"nl{s}") for s in range(NS)]

            # ======== phase 1: theta / phi / g convs (both sides) ========
            for s in range(NS):
                # theta -> A_sb[s]; A^T tile [k, cb] per (b, blk_local)
                for b in range(NB):
                    for blk in range(4):
                        pt = cvtile([P, CB])
                        for ck in range(2):
                            base = (ck * NB + b) * 512 + blk * 128
                            nc.tensor.matmul(
                                pt[:],
                                lhsT=xown_sb[:, base:base + 128],
                                rhs=wth_sb[:, (s * 2 + ck) * CB:(s * 2 + ck + 1) * CB],
                                start=(ck == 0),
                                stop=(ck == 1),
                            )
                        dst = A_sb[s][:, (b * 4 + blk) * CB:(b * 4 + blk + 1) * CB]
                        nc.vector.tensor_tensor(
                            out=dst, in0=pt[:],
                            in1=bthb_sb[:, s * CB:(s + 1) * CB],
                            op=mybir.AluOpType.add,
                        )

                # phi natural conv [cb, hw] per b, then pair-shuffle into P_sb
                for b in range(NB):
                    pn = phin.tile([P, HW], BF16, tag="pn", name="pn")
                    for t in range(8):
                        pt = cvtile([P, 512])
                        for ck in range(2):
                            nc.tensor.matmul(
                                pt[:],
                                lhsT=wph_sb[:, (s * 2 + ck) * CB:(s * 2 + ck + 1) * CB],
                                rhs=xb_sb[:, (ck * NB + b) * HW + t * 512:(ck * NB + b) * HW + (t + 1) * 512],
                                start=(ck == 0),
                                stop=(ck == 1),
                            )
                        nc.scalar.activation(
                            out=pn[:, t * 512:(t + 1) * 512], in_=pt[:],
                            func=AF.Identity, bias=bqkv_sb[:, s * 2:s * 2 + 1],
                        )
                    for sfx in range(2):
                        nc.sync.dma_start(
                            P_sb[s][64 * b:64 * (b + 1), sfx * HW:(sfx + 1) * HW],
                            pn[sfx::2, :],
                        )

                # g conv -> gnat[s] [cb, jc*128 + n], jc = b_g*32 + blkg
                for b in range(NB):
                    for t in range(8):
                        pt = cvtile([P, 512])
                        for ck in range(2):
                            nc.tensor.matmul(
                                pt[:],
                                lhsT=wg_sb[:, (s * 2 + ck) * CB:(s * 2 + ck + 1) * CB],
                                rhs=xb_sb[:, (ck * NB + b) * HW + t * 512:(ck * NB + b) * HW + (t + 1) * 512],
                                start=(ck == 0),
                                stop=(ck == 1),
                            )
                        nc.scalar.activation(
                            out=gnat[s][:, (b * 32 + t * 4) * 128:(b * 32 + t * 4 + 4) * 128],
                            in_=pt[:],
                            func=AF.Identity, bias=bqkv_sb[:, s * 2 + 1:s * 2 + 2],
                        )

            # ---- constants for attention epilogue ----
            ident = const.tile([P, P], BF16, tag="ident", name="ident")
            from concourse.masks import make_identity
            make_identity(nc, ident[:])
            onesb = const.tile([P, 1], BF16, tag="onesb", name="onesb")
            nc.vector.memset(onesb[:], 1.0)
            onesf = const.tile([P, 1], F32, tag="onesf", name="onesf")
            nc.vector.memset(onesf[:], 1.0)

            # ======== attention for both sides (Y^T form) ========
            yhalos = []
            att_tail = [None, None]
            att_mid = [None, None]
            head_tail = [None, None]
            for s in range(NS):
                Pv = P_sb[s].rearrange("p (sh cb bg) -> p sh bg cb", sh=2, cb=CB, bg=32)
                pyt = [ppy.tile([P, 512], F32, tag=f"yt{i}", name=f"yt{s}_{i}") for i in range(NB)]
                pdn = [ppy.tile([1, 512], F32, tag=f"dn{i}", name=f"dn{s}_{i}") for i in range(NB)]
                sbig = [pps.tile([P, 512], F32, tag=f"sb{i}", name=f"sb{s}_{i}") for i in range(3)]
                sts = {}

                def s_mms(k):
                    sphi, blkg = k // 32, k % 32
                    for it in range(NB):
                        nc.tensor.matmul(
                            sbig[(2 * k + it) % 3][:],
                            lhsT=Pv[:, sphi, blkg, :],
                            rhs=A_sb[s][:, it * 512:(it + 1) * 512],
                            start=True, stop=True,
                        )

                def exp_k(k):
                    for it in range(NB):
                        st = stp4.tile([P, 512], BF16, tag="st", name="st")
                        nc.scalar.activation(
                            out=st[:], in_=sbig[(2 * k + it) % 3][:], func=AF.Exp)
                        sts[(k, it)] = st

                def yden(k):
                    for it in range(NB):
                        nc.tensor.matmul(
                            pyt[it][:],
                            lhsT=gnat[s][:, k * 128:(k + 1) * 128],
                            rhs=sts[(k, it)][:],
                            start=(k == 0), stop=(k == 63),
                        )
                    for it in range(NB):
                        mm = nc.tensor.matmul(
                            pdn[it][:1, :],
                            lhsT=onesb[:],
                            rhs=sts.pop((k, it))[:],
                            start=(k == 0), stop=(k == 63),
                        )
                        att_tail[s] = mm
                        if k == 24:
                            att_mid[s] = mm

                s_mms(0)
                s_mms(1)
                exp_k(0)
                for k in range(64):
                    if k + 1 < 64:
                        exp_k(k + 1)
                    if k + 2 < 64:
                        s_mms(k + 2)
                    yden(k)
                # epilogue: transpose Y^T -> ysh, divide by denom
                for it in range(NB):
                    ytsb = f32p.tile([P, 512], BF16, tag="ytsb")
                    nc.vector.tensor_copy(ytsb[:], pyt[it][:])
                    dnsb = smal.tile([1, 512], F32, tag="dnsb")
                    nc.vector.tensor_copy(dnsb[:1, :], pdn[it][:1, :])
                    for gi in range(4):
                        prd = dntile([P, 1])
                        nc.tensor.matmul(
                            prd[:, :1],
                            lhsT=dnsb[0:1, gi * 128:(gi + 1) * 128],
                            rhs=onesf[0:1, 0:1],
                            start=True, stop=True,
                        )
                        rdt = smal.tile([P, 1], F32, tag="rdt")
                        nc.vector.reciprocal(rdt[:], prd[:, :1])
                        ptr = dntile([P, CB], BF16)
                        nc.tensor.transpose(
                            ptr[:], ytsb[:, gi * 128:(gi + 1) * 128], ident[:],
                        )
                        nc.vector.tensor_scalar_mul(
                            ysh[s][:, (it * 4 + gi) * 128:(it * 4 + gi + 1) * 128],
                            ptr[:], rdt[:],
                        )

                # ---- boundary strips -> DRAM -> AllGather -> halo gather ----
                strips = smal.tile([P, 256], BF16, tag="strips")
                for b in range(NB):
                    nc.vector.tensor_copy(
                        strips[:, b * 64:(b + 1) * 64],
                        ysh[s][:, (b * 4 + 0) * 128:(b * 4 + 0) * 128 + 64],
                    )
                    nc.vector.tensor_copy(
                        strips[:, 128 + b * 64:128 + (b + 1) * 64],
                        ysh[s][:, (b * 4 + 3) * 128 + 64:(b * 4 + 3) * 128 + 128],
                    )
                hin = dram.tile([2, P, 128], BF16, tag=f"hin{s}", name=f"hin{s}")
                nc.sync.dma_start(
                    hin[:].rearrange("st p x -> p st x"),
                    strips[:].rearrange("p (st x) -> p st x", st=2),
                )
                hall = dram.tile([NCORES * 2 * P, 128], BF16, tag=f"hall{s}", name=f"hall{s}")
                nc.gpsimd.collective_compute(
                    "AllGather", mybir.AluOpType.bypass,
                    replica_groups=[list(range(NCORES))],
                    ins=[hin.opt()],
                    outs=[hall.opt()],
                )
                yhalo = []
                for strip in range(2):
                    yh = smal.tile([P, 128], BF16, tag=f"yh{strip}", name=f"yh{s}_{strip}")
                    nc.vector.memset(yh[:], 0.0)
                    nc.gpsimd.indirect_dma_start(
                        out=yh[:], out_offset=None,
                        in_=hall[:],
                        in_offset=bass.IndirectOffsetOnAxis(ap=hidx_sb[:, strip:strip + 1], axis=0),
                        bounds_check=NCORES * 2 * P - 1,
                        oob_is_err=False,
                    )
                    yhalo.append(yh)
                yhalos.append(yhalo)

            # ======== per side: W conv + heads ========
            for s in range(NS):
                yhalo = yhalos[s]
                # ---- W conv + bias + residual -> nl_sb[s] (rows 0..9, w-padded 66) ----
                nc.vector.memset(nl_sb[s][:], 0.0)
                nlv = nl_sb[s].rearrange("p (q r w) -> p q r w", q=4, r=10, w=66)
                for b in range(NB):
                    for ck in range(2):
                        lhsT = wW_sb[:, s * C + ck * 128:s * C + (ck + 1) * 128]
                        # own rows (8 rows of 64)
                        pt = cvtile([P, 512])
                        nc.tensor.matmul(
                            pt[:], lhsT=lhsT,
                            rhs=ysh[s][:, b * 512:(b + 1) * 512],
                            start=True, stop=True,
                        )
                        tmp = f32p.tile([P, 512], F32, tag="nlt")
                        nc.vector.tensor_tensor(
                            out=tmp[:], in0=pt[:],
                            in1=xres_sb[:, (ck * NB + b) * 640 + 64:(ck * NB + b) * 640 + 576],
                            op=mybir.AluOpType.add,
                        )
                        nc.scalar.activation(
                            out=nlv[:, ck * NB + b, 1:9, 1:65],
                            in_=tmp[:].rearrange("p (r w) -> p r w", w=64),
                            func=AF.Identity, bias=bW_sb[:, s * 2 + ck:s * 2 + ck + 1],
                        )
                        # halo rows (row 0 and row 9)
                        for strip in range(2):
                            ph = cvtile([P, 64])
                            hmm = nc.tensor.matmul(
                                ph[:], lhsT=lhsT,
                                rhs=yhalo[strip][:, b * 64:(b + 1) * 64],
                                start=True, stop=True,
                            )
                            if s == 0 and att_mid[1] is not None:
                                add_dep_helper(hmm.ins, att_mid[1].ins, sync=False,
                                               reason="halo after other side attn")
                            if s == 1 and head_tail[0] is not None:
                                add_dep_helper(hmm.ins, head_tail[0].ins, sync=False,
                                               reason="s1 halo after s0 heads")
                            tmph = f32p.tile([P, 64], F32, tag="nlh")
                            nc.vector.tensor_tensor(
                                out=tmph[:], in0=ph[:],
                                in1=xres_sb[:, (ck * NB + b) * 640 + strip * 576:(ck * NB + b) * 640 + strip * 576 + 64],
                                op=mybir.AluOpType.add,
                            )
                            tmph2 = f32p.tile([P, 64], F32, tag="nlh2")
                            nc.scalar.activation(
                                out=tmph2[:], in_=tmph[:],
                                func=AF.Identity, bias=bW_sb[:, s * 2 + ck:s * 2 + ck + 1],
                            )
                            nc.vector.tensor_scalar_mul(
                                nlv[:, ck * NB + b, 9 * strip, 1:65],
                                tmph2[:], hmask_sb[:, strip:strip + 1],
                            )

                # ---- heads (0=heat, 1=tag, 2=regr) ----
                for h in range(3):
                    w1t = w1p.tile([P, 9 * 2 * C], BF16, tag="w1t")
                    nc.sync.dma_start(w1t[:], w1_d.ap()[s, h])
                    hs = hp.tile([P, NB * 2 * 512], BF16, tag="hs")
                    for b in range(NB):
                        for cko in range(2):
                            pc = hxtile([P, 512])
                            k = 0
                            for tap in range(9):
                                dy, dx = tap // 3, tap % 3
                                for cki in range(2):
                                    rhs = nlv[:, cki * NB + b, dy:dy + 8, dx:dx + 64]
                                    nc.tensor.matmul(
                                        pc[:],
                                        lhsT=w1t[:, (tap * 2 + cki) * C + cko * 128:(tap * 2 + cki) * C + (cko + 1) * 128],
                                        rhs=rhs,
                                        start=(k == 0), stop=(k == 17),
                                    )
                                    k += 1
                            nc.scalar.activation(
                                out=hs[:, (b * 2 + cko) * 512:(b * 2 + cko + 1) * 512],
                                in_=pc[:], func=AF.Relu,
                                bias=b1_sb[:, (s * 3 + h) * 2 + cko:(s * 3 + h) * 2 + cko + 1],
                            )
                    if h == 0:
                        od, w2_sb, b2_sb, out_d = 80, w2h_sb, b2h_sb, heat_d
                    elif h == 1:
                        od, w2_sb, b2_sb, out_d = 1, w2t_sb, b2t_sb, tago_d
                    else:
                        od, w2_sb, b2_sb, out_d = 2, w2r_sb, b2r_sb, regr_d
                    for b in range(NB):
                        p2 = hxtile([P, 512])
                        for ck in range(2):
                            head_tail[s] = nc.tensor.matmul(
                                p2[:od, :],
                                lhsT=w2_sb[:, (s * 2 + ck) * od:(s * 2 + ck + 1) * od],
                                rhs=hs[:, (b * 2 + ck) * 512:(b * 2 + ck + 1) * 512],
                                start=(ck == 0), stop=(ck == 1),
                            )
                        ob = obp.tile([P, 512], F32, tag="ob")
                        nc.scalar.activation(
                            out=ob[:od, :], in_=p2[:od, :],
                            func=AF.Identity, bias=b2_sb[:od, s:s + 1],
                        )
                        nc.sync.dma_start(out_d.ap()[s, b], ob[:od, :])

    nc.compile()
    return nc


_NC = None


def _get_nc():
    global _NC
    if _NC is None:
        _NC = _build_nc()
    return _NC


def _bf(a):
    return np.ascontiguousarray(np.asarray(a, np.float32).astype(ml_dtypes.bfloat16))


def _f32(a):
    return np.ascontiguousarray(np.asarray(a, dtype=np.float32))


def _prep_inputs(feature, params):
    X = _f32(feature).reshape(NB, C, HW)
    # [cin_p, (cin_ck, b, hw)]
    xb = _bf(X.reshape(NB, 2, 128, HW).transpose(2, 1, 0, 3).reshape(P, 2 * NB * HW))

    sides = ["tl_nl", "br_nl"]
    heads = [["tl_heat", "tl_tag", "tl_regr"], ["br_heat", "br_tag", "br_regr"]]

    wth = np.zeros((P, NS, 2, CB), np.float32)
    wph = np.zeros((P, NS, 2, CB), np.float32)
    wg = np.zeros((P, NS, 2, CB), np.float32)
    wW = np.zeros((P, NS, C), np.float32)
    bthb = np.zeros((P, NS * CB), np.float32)
    bqkv = np.zeros((P, NS, 2), np.float32)
    bW = np.zeros((P, NS, 2), np.float32)
    for s in range(NS):
        p_ = params[sides[s]]
        for arr, key in ((wth, "theta_w"), (wph, "phi_w"), (wg, "g_w")):
            w = _f32(p_[key]).reshape(CB, C)  # [cb, cin]
            wt = w.T.reshape(2, 128, CB)  # [ck, cin_p, cb]
            arr[:, s, :, :] = wt.transpose(1, 0, 2)
        ww = _f32(p_["W_w"]).reshape(C, CB)  # [oc, cb]
        wW[:, s, :] = ww.T  # [cb_p, oc]
        bthb[:, s * CB:(s + 1) * CB] = np.tile(_f32(p_["theta_b"])[None, :], (P, 1))
        bqkv[:, s, 0] = _f32(p_["phi_b"])
        bqkv[:, s, 1] = _f32(p_["g_b"])
        bW[:, s, :] = _f32(p_["W_b"]).reshape(2, 128).T

    w1 = np.zeros((NS, 3, P, 9 * 2 * C), np.float32)
    b1 = np.zeros((P, NS, 3, 2), np.float32)
    w2h = np.zeros((P, NS, 2, 80), np.float32)
    b2h = np.zeros((P, NS), np.float32)
    w2t = np.zeros((P, NS, 2, 1), np.float32)
    b2t = np.zeros((P, NS), np.float32)
    w2r = np.zeros((P, NS, 2, 2), np.float32)
    b2r = np.zeros((P, NS), np.float32)
    for s in range(NS):
        for h in range(3):
            hp_ = params[heads[s][h]]
            wa = _f32(hp_["w1"])  # [oc, ic, 3, 3]
            t = wa.transpose(2, 3, 1, 0).reshape(9, 2, 128, C)  # [tap, ck, ic_p, oc]
            w1[s, h] = t.transpose(2, 0, 1, 3).reshape(P, 9 * 2 * C)
            b1[:, s, h, :] = _f32(hp_["b1"]).reshape(2, 128).T
            w2 = _f32(hp_["w2"])
            od = w2.shape[0]
            w2 = w2.reshape(od, C)
            w2T = w2.T.reshape(2, 128, od).transpose(1, 0, 2)  # [oc_p, ck, od]
            if h == 0:
                w2h[:, s], b2h[:od, s] = w2T, _f32(hp_["b2"])
            elif h == 1:
                w2t[:, s], b2t[:od, s] = w2T, _f32(hp_["b2"])
            else:
                w2r[:, s], b2r[:od, s] = w2T, _f32(hp_["b2"])

    common = {
        "xb": xb,
        "wth": _bf(wth.reshape(P, -1)), "wph": _bf(wph.reshape(P, -1)),
        "wg": _bf(wg.reshape(P, -1)), "wW": _bf(wW.reshape(P, -1)),
        "bthb": bthb, "bqkv": bqkv.reshape(P, -1), "bW": bW.reshape(P, -1),
        "w1": _bf(w1), "b1": b1.reshape(P, -1),
        "w2h": _bf(w2h.reshape(P, -1)), "b2h": b2h,
        "w2t": _bf(w2t.reshape(P, -1)), "b2t": b2t,
        "w2r": _bf(w2r.reshape(P, -1)), "b2r": b2r,
    }

    Ximg = X.reshape(NB, C, H, W)
    in_maps = []
    for d in range(NCORES):
        m = dict(common)
        xo = X[:, :, d * 512:(d + 1) * 512]  # [b, cin, 512]
        m["xown"] = _bf(xo.reshape(NB, 2, 128, 512).transpose(2, 1, 0, 3).reshape(P, -1))
        xr = np.zeros((NB, C, 10, W), np.float32)
        r0, r1 = 8 * d - 1, 8 * d + 9
        rr0, rr1 = max(r0, 0), min(r1, H)
        xr[:, :, rr0 - r0:rr0 - r0 + (rr1 - rr0), :] = Ximg[:, :, rr0:rr1, :]
        m["xres"] = np.ascontiguousarray(
            xr.reshape(NB, 2, 128, 640).transpose(2, 1, 0, 3).reshape(P, -1))
        hidx = np.full((P, 2), NCORES * 2 * P, np.int32)
        if d > 0:
            hidx[:, 0] = (d - 1) * 256 + 128 + np.arange(P)
        if d < NCORES - 1:
            hidx[:, 1] = (d + 1) * 256 + np.arange(P)
        m["hidx"] = hidx
        hm = np.zeros((P, 2), np.float32)
        hm[:, 0] = 1.0 if d > 0 else 0.0
        hm[:, 1] = 1.0 if d < NCORES - 1 else 0.0
        m["hmask"] = hm
        in_maps.append(m)
    return in_maps


def _gather_feat(feat, ind):
    B, Cc, Hh, Ww = feat.shape
    f = feat.transpose(0, 2, 3, 1).reshape(B, Hh * Ww, Cc)
    return np.take_along_axis(f, np.asarray(ind)[:, :, None].astype(np.int64), axis=1)


def kernel(feature, tl_tags, br_tags, params):
    nc = _get_nc()
    in_maps = _prep_inputs(feature, params)
    res = run_bass_kernel_spmd(nc, in_maps, core_ids=list(range(NCORES)))
    rs = res.results

    def assemble(key, od):
        full = np.zeros((NS, NB, od, H, W), np.float32)
        for d in range(NCORES):
            full[:, :, :, 8 * d:8 * (d + 1), :] = rs[d][key].reshape(NS, NB, od, 8, W)
        return full

    heat = assemble("heat", 80)
    tagm = assemble("tago", 1)
    regm = assemble("regr", 2)

    tl_heat, br_heat = heat[0], heat[1]
    tl_tag = _gather_feat(tagm[0], tl_tags)
    br_tag = _gather_feat(tagm[1], br_tags)
    tl_regr = _gather_feat(regm[0], tl_tags)
    br_regr = _gather_feat(regm[1], br_tags)
    return (tl_heat, br_heat, tl_tag, br_tag, tl_regr, br_regr)


# revision 22
# speedup vs baseline: 1.1381x; 1.1381x over previous
"""CornerNet module (2x nonlocal attention + 6 conv heads) on 8 Trainium2 cores.

Sharding: the NxN (8192x8192) attention in each nonlocal block is sharded by
rows across the 8 cores.  Row-group granularity is 128 rows; row i of the
attention corresponds to flat index (b, cb, blk) of the theta buffer, and we
assign core d the blocks blk in [4d, 4d+4) for both batches.  Those blocks map
to image rows [8d, 8d+8) of the y output, so each core also owns a contiguous
spatial slice for the conv heads.  phi / g / the 1x1 convs are computed
replicated (they are tiny).  The 1-row halo needed by the 3x3 head convs is
exchanged with an AllGather of the boundary y rows.

Layout cheat sheet (per side):
  A   = theta flat  [8192, 128]  rows i=(b, cb, blk), col k = hw offset in blk
  P   = phi  flat   [128, 8192]  row k=(b_k, cb//2),  col j=(cb%2)*4096+hw
  G   = g    flat   [8192, 128]  row j, col n;  G[(s,u), n] = g[s, u//32, (u%32)*128+n]
  S^T tile for j-chunk (s, blkg):  lhsT = P[:, s*4096+blkg :: 32]  (cb on M)
                                   rhs  = A^T tile [k, i]          (from theta conv)
  Y accumulation: lhsT = exp(S^T)[:, i-slice], rhs = g_nat[s][:, blkg*128:+128] | ones
Softmax skips max-subtraction: |scores| <= |theta_row| * |phi_col| ~ 15, safe in f32.
"""

import numpy as np
import ml_dtypes

import concourse.bass as bass
import concourse.mybir as mybir
import concourse.tile as tile
from concourse.tile import add_dep_helper
from concourse import bacc
from concourse.bass_utils import run_bass_kernel_spmd

BF16 = mybir.dt.bfloat16
F32 = mybir.dt.float32
I32 = mybir.dt.int32

P = 128          # partitions
NB = 2           # batch
NS = 2           # sides (tl, br)
C = 256          # channels
CB = 128         # bottleneck channels
H = W = 64
HW = H * W       # 4096
NCORES = 8
AF = mybir.ActivationFunctionType


def _build_nc():
    nc = bacc.Bacc(num_devices=NCORES)

    # ---- inputs (all 2D except w1) ----
    xb_d = nc.dram_tensor("xb", [P, 2 * NB * HW], BF16, kind="ExternalInput")
    xown_d = nc.dram_tensor("xown", [P, 2 * NB * 512], BF16, kind="ExternalInput")
    xres_d = nc.dram_tensor("xres", [P, 2 * NB * 640], F32, kind="ExternalInput")
    wth_d = nc.dram_tensor("wth", [P, NS * 2 * CB], BF16, kind="ExternalInput")
    wph_d = nc.dram_tensor("wph", [P, NS * 2 * CB], BF16, kind="ExternalInput")
    wg_d = nc.dram_tensor("wg", [P, NS * 2 * CB], BF16, kind="ExternalInput")
    wW_d = nc.dram_tensor("wW", [P, NS * C], BF16, kind="ExternalInput")
    bthb_d = nc.dram_tensor("bthb", [P, NS * CB], F32, kind="ExternalInput")
    bqkv_d = nc.dram_tensor("bqkv", [P, NS * 2], F32, kind="ExternalInput")  # phi,g
    bW_d = nc.dram_tensor("bW", [P, NS * 2], F32, kind="ExternalInput")
    w1_d = nc.dram_tensor("w1", [NS, 3, P, 9 * 2 * C], BF16, kind="ExternalInput")
    b1_d = nc.dram_tensor("b1", [P, NS * 3 * 2], F32, kind="ExternalInput")
    w2h_d = nc.dram_tensor("w2h", [P, NS * 2 * 80], BF16, kind="ExternalInput")
    b2h_d = nc.dram_tensor("b2h", [P, NS], F32, kind="ExternalInput")
    w2t_d = nc.dram_tensor("w2t", [P, NS * 2 * 1], BF16, kind="ExternalInput")
    b2t_d = nc.dram_tensor("b2t", [P, NS], F32, kind="ExternalInput")
    w2r_d = nc.dram_tensor("w2r", [P, NS * 2 * 2], BF16, kind="ExternalInput")
    b2r_d = nc.dram_tensor("b2r", [P, NS], F32, kind="ExternalInput")
    hidx_d = nc.dram_tensor("hidx", [P, 2], I32, kind="ExternalInput")
    hmask_d = nc.dram_tensor("hmask", [P, 2], F32, kind="ExternalInput")

    # ---- outputs (spatial shard: rows [8d, 8d+8) = 512 pixels per batch) ----
    heat_d = nc.dram_tensor("heat", [NS, NB, 80, 512], F32, kind="ExternalOutput")
    tago_d = nc.dram_tensor("tago", [NS, NB, 1, 512], F32, kind="ExternalOutput")
    regr_d = nc.dram_tensor("regr", [NS, NB, 2, 512], F32, kind="ExternalOutput")

    with tile.TileContext(nc) as tc:
        with (
            tc.tile_pool(name="const", bufs=1) as const,
            tc.tile_pool(name="pers", bufs=1) as pers,
            tc.tile_pool(name="phin", bufs=2) as phin,
            tc.tile_pool(name="stp", bufs=3) as stp,
            tc.tile_pool(name="w1p", bufs=2) as w1p,
            tc.tile_pool(name="hp", bufs=2) as hp,
            tc.tile_pool(name="f32p", bufs=2) as f32p,
            tc.tile_pool(name="obp", bufs=2) as obp,
            tc.tile_pool(name="smal", bufs=4) as smal,
            tc.tile_pool(name="py", bufs=1, space="PSUM") as ppy,
            tc.tile_pool(name="ps4", bufs=1, space="PSUM") as pps,
            tc.tile_pool(name="st4", bufs=4) as stp4,
            tc.tile_pool(name="dram", bufs=1, space="DRAM") as dram,
        ):
            _cvstate = [0]
            _cv4tags = ["yt0", "yt1", "dn0", "dn1"]

            def cvtile(shape):
                _cvstate[0] ^= 1
                return ppy.tile(shape, F32, tag=f"yt{_cvstate[0]}", name="cvt")

            def cvtile4(shape):
                _cvstate[0] = (_cvstate[0] + 1) % 4
                return ppy.tile(shape, F32, tag=_cv4tags[_cvstate[0]], name="cv4")

            def dntile(shape, dtype=F32):
                _cvstate[0] ^= 1
                return ppy.tile(shape, dtype, tag=f"dn{_cvstate[0]}", name="dnt")

            # ---- load inputs into SBUF ----
            def load(d, dtype, tag):
                t = const.tile(list(d.shape), dtype, tag=tag, name=tag)
                nc.sync.dma_start(t[:], d.ap())
                return t

            wth_sb = load(wth_d, BF16, "wth")
            xown_sb = load(xown_d, BF16, "xown")
            wph_sb = load(wph_d, BF16, "wph")
            wg_sb = load(wg_d, BF16, "wg")
            xb_sb = const.tile(list(xb_d.shape), BF16, tag="xb", name="xb")
            for b_ in range(NB):
                for h_ in range(2):
                    for ck_ in range(2):
                        o_ = (ck_ * NB + b_) * HW + h_ * 2048
                        nc.sync.dma_start(xb_sb[:, o_:o_ + 2048], xb_d.ap()[:, o_:o_ + 2048])
            wW_sb = load(wW_d, BF16, "wW")
            bthb_sb = load(bthb_d, F32, "bthb")
            bqkv_sb = load(bqkv_d, F32, "bqkv")
            bW_sb = load(bW_d, F32, "bW")
            b1_sb = load(b1_d, F32, "b1")
            w2h_sb = load(w2h_d, BF16, "w2h")
            b2h_sb = load(b2h_d, F32, "b2h")
            w2t_sb = load(w2t_d, BF16, "w2t")
            b2t_sb = load(b2t_d, F32, "b2t")
            w2r_sb = load(w2r_d, BF16, "w2r")
            b2r_sb = load(b2r_d, F32, "b2r")
            hidx_sb = load(hidx_d, I32, "hidx")
            hmask_sb = load(hmask_d, F32, "hmask")
            xres_sb = load(xres_d, F32, "xres")

            # persistent per-side tensors
            P_sb = [pers.tile([P, 2 * HW], BF16, tag=f"P{s}", name=f"P{s}") for s in range(NS)]
            gnat = [pers.tile([P, 64 * 128], BF16, tag=f"g{s}", name=f"g{s}") for s in range(NS)]
            A_sb = [pers.tile([P, 8 * CB], BF16, tag=f"A{s}", name=f"A{s}") for s in range(NS)]
            ysh = [pers.tile([P, 8 * CB], BF16, tag=f"y{s}", name=f"ysh{s}") for s in range(NS)]
            nl_sb = [pers.tile([P, 4 * 10 * 66], BF16, tag=f"nl{s}", name=f"nl{s}") for s in range(NS)]

            # ======== phase 1: theta / phi / g convs (both sides) ========
            for s in range(NS):
                # theta -> A_sb[s]; A^T tile [k, cb] per (b, blk_local)
                for b in range(NB):
                    for blk in range(4):
                        pt = cvtile([P, CB])
                        for ck in range(2):
                            base = (ck * NB + b) * 512 + blk * 128
                            nc.tensor.matmul(
                                pt[:],
                                lhsT=xown_sb[:, base:base + 128],
                                rhs=wth_sb[:, (s * 2 + ck) * CB:(s * 2 + ck + 1) * CB],
                                start=(ck == 0),
                                stop=(ck == 1),
                            )
                        dst = A_sb[s][:, (b * 4 + blk) * CB:(b * 4 + blk + 1) * CB]
                        nc.vector.tensor_tensor(
                            out=dst, in0=pt[:],
                            in1=bthb_sb[:, s * CB:(s + 1) * CB],
                            op=mybir.AluOpType.add,
                        )

                # phi natural conv [cb, hw] per b, then pair-shuffle into P_sb
                for b in range(NB):
                    pn = phin.tile([P, HW], BF16, tag="pn", name="pn")
                    for t in range(8):
                        pt = cvtile([P, 512])
                        for ck in range(2):
                            nc.tensor.matmul(
                                pt[:],
                                lhsT=wph_sb[:, (s * 2 + ck) * CB:(s * 2 + ck + 1) * CB],
                                rhs=xb_sb[:, (ck * NB + b) * HW + t * 512:(ck * NB + b) * HW + (t + 1) * 512],
                                start=(ck == 0),
                                stop=(ck == 1),
                            )
                        nc.scalar.activation(
                            out=pn[:, t * 512:(t + 1) * 512], in_=pt[:],
                            func=AF.Identity, bias=bqkv_sb[:, s * 2:s * 2 + 1],
                        )
                    for sfx in range(2):
                        nc.sync.dma_start(
                            P_sb[s][64 * b:64 * (b + 1), sfx * HW:(sfx + 1) * HW],
                            pn[sfx::2, :],
                        )

                # g conv -> gnat[s] [cb, jc*128 + n], jc = b_g*32 + blkg
                for b in range(NB):
                    for t in range(8):
                        pt = cvtile([P, 512])
                        for ck in range(2):
                            nc.tensor.matmul(
                                pt[:],
                                lhsT=wg_sb[:, (s * 2 + ck) * CB:(s * 2 + ck + 1) * CB],
                                rhs=xb_sb[:, (ck * NB + b) * HW + t * 512:(ck * NB + b) * HW + (t + 1) * 512],
                                start=(ck == 0),
                                stop=(ck == 1),
                            )
                        nc.scalar.activation(
                            out=gnat[s][:, (b * 32 + t * 4) * 128:(b * 32 + t * 4 + 4) * 128],
                            in_=pt[:],
                            func=AF.Identity, bias=bqkv_sb[:, s * 2 + 1:s * 2 + 2],
                        )

            # ---- constants for attention epilogue ----
            ident = const.tile([P, P], BF16, tag="ident", name="ident")
            from concourse.masks import make_identity
            make_identity(nc, ident[:])
            onesb = const.tile([P, 1], BF16, tag="onesb", name="onesb")
            nc.vector.memset(onesb[:], 1.0)
            onesf = const.tile([P, 1], F32, tag="onesf", name="onesf")
            nc.vector.memset(onesf[:], 1.0)

            # ======== attention for both sides (Y^T form) ========
            yhalos = []
            att_tail = [None, None]
            att_mid = [None, None]
            head_tail = [None, None]
            for s in range(NS):
                Pv = P_sb[s].rearrange("p (sh cb bg) -> p sh bg cb", sh=2, cb=CB, bg=32)
                pyt = [ppy.tile([P, 512], F32, tag=f"yt{i}", name=f"yt{s}_{i}") for i in range(NB)]
                pdn = [ppy.tile([1, 512], F32, tag=f"dn{i}", name=f"dn{s}_{i}") for i in range(NB)]
                sbig = [pps.tile([P, 1024], F32, tag=f"sb{i}", name=f"sb{s}_{i}") for i in range(2)]
                sts = {}

                def s_mms(k):
                    sphi, blkg = k // 32, k % 32
                    for it in range(NB):
                        nc.tensor.matmul(
                            sbig[k % 2][:, it * 512:(it + 1) * 512],
                            lhsT=Pv[:, sphi, blkg, :],
                            rhs=A_sb[s][:, it * 512:(it + 1) * 512],
                            start=True, stop=True,
                        )

                def exp_k(k):
                    st = stp4.tile([P, 1024], BF16, tag="st", name="st")
                    nc.scalar.activation(
                        out=st[:], in_=sbig[k % 2][:], func=AF.Exp)
                    sts[k] = st

                def yden(k):
                    st = sts.pop(k)
                    for it in range(NB):
                        nc.tensor.matmul(
                            pyt[it][:],
                            lhsT=gnat[s][:, k * 128:(k + 1) * 128],
                            rhs=st[:, it * 512:(it + 1) * 512],
                            start=(k == 0), stop=(k == 63),
                        )
                    for it in range(NB):
                        mm = nc.tensor.matmul(
                            pdn[it][:1, :],
                            lhsT=onesb[:],
                            rhs=st[:, it * 512:(it + 1) * 512],
                            start=(k == 0), stop=(k == 63),
                        )
                        att_tail[s] = mm
                        if k == 24:
                            att_mid[s] = mm

                s_mms(0)
                s_mms(1)
                exp_k(0)
                for k in range(64):
                    if k + 1 < 64:
                        exp_k(k + 1)
                    if k + 2 < 64:
                        s_mms(k + 2)
                    yden(k)
                # epilogue: transpose Y^T -> ysh, divide by denom
                for it in range(NB):
                    ytsb = f32p.tile([P, 512], BF16, tag="ytsb")
                    nc.vector.tensor_copy(ytsb[:], pyt[it][:])
                    dnsb = smal.tile([1, 512], F32, tag="dnsb")
                    nc.vector.tensor_copy(dnsb[:1, :], pdn[it][:1, :])
                    for gi in range(4):
                        prd = dntile([P, 1])
                        nc.tensor.matmul(
                            prd[:, :1],
                            lhsT=dnsb[0:1, gi * 128:(gi + 1) * 128],
                            rhs=onesf[0:1, 0:1],
                            start=True, stop=True,
                        )
                        rdt = smal.tile([P, 1], F32, tag="rdt")
                        nc.vector.reciprocal(rdt[:], prd[:, :1])
                        ptr = dntile([P, CB], BF16)
                        nc.tensor.transpose(
                            ptr[:], ytsb[:, gi * 128:(gi + 1) * 128], ident[:],
                        )
                        nc.vector.tensor_scalar_mul(
                            ysh[s][:, (it * 4 + gi) * 128:(it * 4 + gi + 1) * 128],
                            ptr[:], rdt[:],
                        )

                # ---- boundary strips -> DRAM -> AllGather -> halo gather ----
                strips = smal.tile([P, 256], BF16, tag="strips")
                for b in range(NB):
                    nc.vector.tensor_copy(
                        strips[:, b * 64:(b + 1) * 64],
                        ysh[s][:, (b * 4 + 0) * 128:(b * 4 + 0) * 128 + 64],
                    )
                    nc.vector.tensor_copy(
                        strips[:, 128 + b * 64:128 + (b + 1) * 64],
                        ysh[s][:, (b * 4 + 3) * 128 + 64:(b * 4 + 3) * 128 + 128],
                    )
                hin = dram.tile([2, P, 128], BF16, tag=f"hin{s}", name=f"hin{s}")
                nc.sync.dma_start(
                    hin[:].rearrange("st p x -> p st x"),
                    strips[:].rearrange("p (st x) -> p st x", st=2),
                )
                hall = dram.tile([NCORES * 2 * P, 128], BF16, tag=f"hall{s}", name=f"hall{s}")
                nc.gpsimd.collective_compute(
                    "AllGather", mybir.AluOpType.bypass,
                    replica_groups=[list(range(NCORES))],
                    ins=[hin.opt()],
                    outs=[hall.opt()],
                )
                yhalo = []
                for strip in range(2):
                    yh = smal.tile([P, 128], BF16, tag=f"yh{strip}", name=f"yh{s}_{strip}")
                    nc.vector.memset(yh[:], 0.0)
                    nc.gpsimd.indirect_dma_start(
                        out=yh[:], out_offset=None,
                        in_=hall[:],
                        in_offset=bass.IndirectOffsetOnAxis(ap=hidx_sb[:, strip:strip + 1], axis=0),
                        bounds_check=NCORES * 2 * P - 1,
                        oob_is_err=False,
                    )
                    yhalo.append(yh)
                yhalos.append(yhalo)

            # ======== per side: W conv + heads ========
            for s in range(NS):
                yhalo = yhalos[s]
                # ---- W conv + bias + residual -> nl_sb[s] (rows 0..9, w-padded 66) ----
                nc.vector.memset(nl_sb[s][:], 0.0)
                nlv = nl_sb[s].rearrange("p (q r w) -> p q r w", q=4, r=10, w=66)
                for b in range(NB):
                    for ck in range(2):
                        lhsT = wW_sb[:, s * C + ck * 128:s * C + (ck + 1) * 128]
                        # own rows (8 rows of 64)
                        pt = cvtile([P, 512])
                        nc.tensor.matmul(
                            pt[:], lhsT=lhsT,
                            rhs=ysh[s][:, b * 512:(b + 1) * 512],
                            start=True, stop=True,
                        )
                        tmp = f32p.tile([P, 512], F32, tag="nlt")
                        nc.vector.tensor_tensor(
                            out=tmp[:], in0=pt[:],
                            in1=xres_sb[:, (ck * NB + b) * 640 + 64:(ck * NB + b) * 640 + 576],
                            op=mybir.AluOpType.add,
                        )
                        nc.scalar.activation(
                            out=nlv[:, ck * NB + b, 1:9, 1:65],
                            in_=tmp[:].rearrange("p (r w) -> p r w", w=64),
                            func=AF.Identity, bias=bW_sb[:, s * 2 + ck:s * 2 + ck + 1],
                        )
                        # halo rows (row 0 and row 9)
                        for strip in range(2):
                            ph = cvtile([P, 64])
                            hmm = nc.tensor.matmul(
                                ph[:], lhsT=lhsT,
                                rhs=yhalo[strip][:, b * 64:(b + 1) * 64],
                                start=True, stop=True,
                            )
                            if s == 0 and att_mid[1] is not None:
                                add_dep_helper(hmm.ins, att_mid[1].ins, sync=False,
                                               reason="halo after other side attn")
                            if s == 1 and head_tail[0] is not None:
                                add_dep_helper(hmm.ins, head_tail[0].ins, sync=False,
                                               reason="s1 halo after s0 heads")
                            tmph = f32p.tile([P, 64], F32, tag="nlh")
                            nc.vector.tensor_tensor(
                                out=tmph[:], in0=ph[:],
                                in1=xres_sb[:, (ck * NB + b) * 640 + strip * 576:(ck * NB + b) * 640 + strip * 576 + 64],
                                op=mybir.AluOpType.add,
                            )
                            tmph2 = f32p.tile([P, 64], F32, tag="nlh2")
                            nc.scalar.activation(
                                out=tmph2[:], in_=tmph[:],
                                func=AF.Identity, bias=bW_sb[:, s * 2 + ck:s * 2 + ck + 1],
                            )
                            nc.vector.tensor_scalar_mul(
                                nlv[:, ck * NB + b, 9 * strip, 1:65],
                                tmph2[:], hmask_sb[:, strip:strip + 1],
                            )

                # ---- heads (0=heat, 1=tag, 2=regr) ----
                for h in range(3):
                    w1t = w1p.tile([P, 9 * 2 * C], BF16, tag="w1t")
                    nc.sync.dma_start(w1t[:], w1_d.ap()[s, h])
                    hs = hp.tile([P, NB * 2 * 512], BF16, tag="hs")
                    for b in range(NB):
                        for cko in range(2):
                            pc = cvtile([P, 512])
                            k = 0
                            for tap in range(9):
                                dy, dx = tap // 3, tap % 3
                                for cki in range(2):
                                    rhs = nlv[:, cki * NB + b, dy:dy + 8, dx:dx + 64]
                                    nc.tensor.matmul(
                                        pc[:],
                                        lhsT=w1t[:, (tap * 2 + cki) * C + cko * 128:(tap * 2 + cki) * C + (cko + 1) * 128],
                                        rhs=rhs,
                                        start=(k == 0), stop=(k == 17),
                                    )
                                    k += 1
                            nc.scalar.activation(
                                out=hs[:, (b * 2 + cko) * 512:(b * 2 + cko + 1) * 512],
                                in_=pc[:], func=AF.Relu,
                                bias=b1_sb[:, (s * 3 + h) * 2 + cko:(s * 3 + h) * 2 + cko + 1],
                            )
                    if h == 0:
                        od, w2_sb, b2_sb, out_d = 80, w2h_sb, b2h_sb, heat_d
                    elif h == 1:
                        od, w2_sb, b2_sb, out_d = 1, w2t_sb, b2t_sb, tago_d
                    else:
                        od, w2_sb, b2_sb, out_d = 2, w2r_sb, b2r_sb, regr_d
                    for b in range(NB):
                        p2 = cvtile([P, 512])
                        for ck in range(2):
                            head_tail[s] = nc.tensor.matmul(
                                p2[:od, :],
                                lhsT=w2_sb[:, (s * 2 + ck) * od:(s * 2 + ck + 1) * od],
                                rhs=hs[:, (b * 2 + ck) * 512:(b * 2 + ck + 1) * 512],
                                start=(ck == 0), stop=(ck == 1),
                            )
                        ob = obp.tile([P, 512], F32, tag="ob")
                        nc.scalar.activation(
                            out=ob[:od, :], in_=p2[:od, :],
                            func=AF.Identity, bias=b2_sb[:od, s:s + 1],
                        )
                        nc.sync.dma_start(out_d.ap()[s, b], ob[:od, :])

    nc.compile()
    return nc


_NC = None


def _get_nc():
    global _NC
    if _NC is None:
        _NC = _build_nc()
    return _NC


def _bf(a):
    return np.ascontiguousarray(np.asarray(a, np.float32).astype(ml_dtypes.bfloat16))


def _f32(a):
    return np.ascontiguousarray(np.asarray(a, dtype=np.float32))


def _prep_inputs(feature, params):
    X = _f32(feature).reshape(NB, C, HW)
    # [cin_p, (cin_ck, b, hw)]
    xb = _bf(X.reshape(NB, 2, 128, HW).transpose(2, 1, 0, 3).reshape(P, 2 * NB * HW))

    sides = ["tl_nl", "br_nl"]
    heads = [["tl_heat", "tl_tag", "tl_regr"], ["br_heat", "br_tag", "br_regr"]]

    wth = np.zeros((P, NS, 2, CB), np.float32)
    wph = np.zeros((P, NS, 2, CB), np.float32)
    wg = np.zeros((P, NS, 2, CB), np.float32)
    wW = np.zeros((P, NS, C), np.float32)
    bthb = np.zeros((P, NS * CB), np.float32)
    bqkv = np.zeros((P, NS, 2), np.float32)
    bW = np.zeros((P, NS, 2), np.float32)
    for s in range(NS):
        p_ = params[sides[s]]
        for arr, key in ((wth, "theta_w"), (wph, "phi_w"), (wg, "g_w")):
            w = _f32(p_[key]).reshape(CB, C)  # [cb, cin]
            wt = w.T.reshape(2, 128, CB)  # [ck, cin_p, cb]
            arr[:, s, :, :] = wt.transpose(1, 0, 2)
        ww = _f32(p_["W_w"]).reshape(C, CB)  # [oc, cb]
        wW[:, s, :] = ww.T  # [cb_p, oc]
        bthb[:, s * CB:(s + 1) * CB] = np.tile(_f32(p_["theta_b"])[None, :], (P, 1))
        bqkv[:, s, 0] = _f32(p_["phi_b"])
        bqkv[:, s, 1] = _f32(p_["g_b"])
        bW[:, s, :] = _f32(p_["W_b"]).reshape(2, 128).T

    w1 = np.zeros((NS, 3, P, 9 * 2 * C), np.float32)
    b1 = np.zeros((P, NS, 3, 2), np.float32)
    w2h = np.zeros((P, NS, 2, 80), np.float32)
    b2h = np.zeros((P, NS), np.float32)
    w2t = np.zeros((P, NS, 2, 1), np.float32)
    b2t = np.zeros((P, NS), np.float32)
    w2r = np.zeros((P, NS, 2, 2), np.float32)
    b2r = np.zeros((P, NS), np.float32)
    for s in range(NS):
        for h in range(3):
            hp_ = params[heads[s][h]]
            wa = _f32(hp_["w1"])  # [oc, ic, 3, 3]
            t = wa.transpose(2, 3, 1, 0).reshape(9, 2, 128, C)  # [tap, ck, ic_p, oc]
            w1[s, h] = t.transpose(2, 0, 1, 3).reshape(P, 9 * 2 * C)
            b1[:, s, h, :] = _f32(hp_["b1"]).reshape(2, 128).T
            w2 = _f32(hp_["w2"])
            od = w2.shape[0]
            w2 = w2.reshape(od, C)
            w2T = w2.T.reshape(2, 128, od).transpose(1, 0, 2)  # [oc_p, ck, od]
            if h == 0:
                w2h[:, s], b2h[:od, s] = w2T, _f32(hp_["b2"])
            elif h == 1:
                w2t[:, s], b2t[:od, s] = w2T, _f32(hp_["b2"])
            else:
                w2r[:, s], b2r[:od, s] = w2T, _f32(hp_["b2"])

    common = {
        "xb": xb,
        "wth": _bf(wth.reshape(P, -1)), "wph": _bf(wph.reshape(P, -1)),
        "wg": _bf(wg.reshape(P, -1)), "wW": _bf(wW.reshape(P, -1)),
        "bthb": bthb, "bqkv": bqkv.reshape(P, -1), "bW": bW.reshape(P, -1),
        "w1": _bf(w1), "b1": b1.reshape(P, -1),
        "w2h": _bf(w2h.reshape(P, -1)), "b2h": b2h,
        "w2t": _bf(w2t.reshape(P, -1)), "b2t": b2t,
        "w2r": _bf(w2r.reshape(P, -1)), "b2r": b2r,
    }

    Ximg = X.reshape(NB, C, H, W)
    in_maps = []
    for d in range(NCORES):
        m = dict(common)
        xo = X[:, :, d * 512:(d + 1) * 512]  # [b, cin, 512]
        m["xown"] = _bf(xo.reshape(NB, 2, 128, 512).transpose(2, 1, 0, 3).reshape(P, -1))
        xr = np.zeros((NB, C, 10, W), np.float32)
        r0, r1 = 8 * d - 1, 8 * d + 9
        rr0, rr1 = max(r0, 0), min(r1, H)
        xr[:, :, rr0 - r0:rr0 - r0 + (rr1 - rr0), :] = Ximg[:, :, rr0:rr1, :]
        m["xres"] = np.ascontiguousarray(
            xr.reshape(NB, 2, 128, 640).transpose(2, 1, 0, 3).reshape(P, -1))
        hidx = np.full((P, 2), NCORES * 2 * P, np.int32)
        if d > 0:
            hidx[:, 0] = (d - 1) * 256 + 128 + np.arange(P)
        if d < NCORES - 1:
            hidx[:, 1] = (d + 1) * 256 + np.arange(P)
        m["hidx"] = hidx
        hm = np.zeros((P, 2), np.float32)
        hm[:, 0] = 1.0 if d > 0 else 0.0
        hm[:, 1] = 1.0 if d < NCORES - 1 else 0.0
        m["hmask"] = hm
        in_maps.append(m)
    return in_maps


def _gather_feat(feat, ind):
    B, Cc, Hh, Ww = feat.shape
    f = feat.transpose(0, 2, 3, 1).reshape(B, Hh * Ww, Cc)
    return np.take_along_axis(f, np.asarray(ind)[:, :, None].astype(np.int64), axis=1)


def kernel(feature, tl_tags, br_tags, params):
    nc = _get_nc()
    in_maps = _prep_inputs(feature, params)
    res = run_bass_kernel_spmd(nc, in_maps, core_ids=list(range(NCORES)))
    rs = res.results

    def assemble(key, od):
        full = np.zeros((NS, NB, od, H, W), np.float32)
        for d in range(NCORES):
            full[:, :, :, 8 * d:8 * (d + 1), :] = rs[d][key].reshape(NS, NB, od, 8, W)
        return full

    heat = assemble("heat", 80)
    tagm = assemble("tago", 1)
    regm = assemble("regr", 2)

    tl_heat, br_heat = heat[0], heat[1]
    tl_tag = _gather_feat(tagm[0], tl_tags)
    br_tag = _gather_feat(tagm[1], br_tags)
    tl_regr = _gather_feat(regm[0], tl_tags)
    br_regr = _gather_feat(regm[1], br_tags)
    return (tl_heat, br_heat, tl_tag, br_tag, tl_regr, br_regr)
